# revision 16
# baseline (speedup 1.0000x reference)
"""Self-contained Trainium2 Bass kernel for nn_CAELoss (loss_fn).

Contract: kernel(**inputs) takes the FULL unsharded inputs
(x [4096,3072], x_hat [4096,3072], target [4096] i32, z_in [4096,128],
z_out [4096,128], center_arr [10,128]) and returns the FULL output
(scalar f32 loss).

Strategy (data-parallel over batch, 8 NeuronCores):
  - each core gets 512 batch rows of x/x_hat/z_in/z_out (+ host-built
    one-hot masks of target), plus the replicated (host-normalized)
    centers.
  - on-device per core: partial sums of (x-x_hat)^2 [dominant, 12 MiB
    of DMA per core], triplet-center terms, outlier terms, and the
    orthogonality residual (gram - I) row sums.
  - device emits a [128, 17] tile of per-partition partial sums; host
    reduces the 8x128 partials to the scalar loss (replaces the
    all-reduce of scalar partial losses).
"""

import sys

import numpy as np

if "/opt/trn_rl_repo" not in sys.path:
    sys.path.insert(0, "/opt/trn_rl_repo")

B, D, C, L = 4096, 3072, 10, 128
N_CORES = 8
BS = B // N_CORES  # 512 batch rows per core
P = 128  # SBUF partitions
NT = BS // P  # 4 z-tiles of 128 rows per core
# MSE chunk layout: (row-tile, col offset, width); final row-tile tapers
# so the post-stream compute tail is short.
MSE_CHUNKS = [
    (0, 0, 1536), (0, 1536, 1536),
    (1, 0, 1536), (1, 1536, 1536),
    (2, 0, 1536), (2, 1536, 1536),
    (3, 0, 1536), (3, 1536, 768), (3, 2304, 768),
]
NCH = len(MSE_CHUNKS)
ZF = 3 * L + 2 * C  # fused z-row: z_nat | z_tr | zo_nat | oh | bm
NSTAT = NCH + NT + NT + 1  # stats columns: mse | tc | outlier | orth
D_IN = 0.1
D_OUT = 1.0
BIG = 1.0e9

ALL_PARTS = frozenset({"mse", "orth", "triplet", "outlier"})

_CACHE = {}


def _build(parts=ALL_PARTS):
    """Build + compile the single-core SPMD Bass program."""
    from contextlib import ExitStack

    import concourse.bacc as bacc
    import concourse.mybir as mybir
    import concourse.tile as tile

    f32 = mybir.dt.float32
    Alu = mybir.AluOpType
    Act = mybir.ActivationFunctionType

    nc = bacc.Bacc(
        "TRN2",
        target_bir_lowering=False,
        debug=False,
        enable_asserts=True,
        num_devices=N_CORES,
    )

    x_d = nc.dram_tensor("x", [BS, D], f32, kind="ExternalInput")
    xh_d = nc.dram_tensor("x_hat", [BS, D], f32, kind="ExternalInput")
    zf_d = nc.dram_tensor("zfused", [P, NT, ZF], f32, kind="ExternalInput")
    ct_d = nc.dram_tensor("cen_t", [L, C], f32, kind="ExternalInput")
    out_d = nc.dram_tensor("out", [P, NSTAT], f32, kind="ExternalOutput")

    eye10_d = nc.inline_tensor(np.eye(C, dtype=np.float32), "eye10")

    # chunk j -> (row-tile, col) slice of x/x_hat
    def chunk(td, j):
        r, c0, w = MSE_CHUNKS[j]
        return td[r * P : (r + 1) * P, c0 : c0 + w]

    with tile.TileContext(nc) as tc, ExitStack() as ctx:
        xp = ctx.enter_context(tc.tile_pool(name="xp", bufs=NCH))
        xhp = ctx.enter_context(tc.tile_pool(name="xhp", bufs=NCH))
        dfp = ctx.enter_context(tc.tile_pool(name="dfp", bufs=3))
        sqp = ctx.enter_context(tc.tile_pool(name="sqp", bufs=3))
        sp = ctx.enter_context(tc.tile_pool(name="sp", bufs=3))
        st = ctx.enter_context(tc.tile_pool(name="st", bufs=1))
        pp = ctx.enter_context(tc.tile_pool(name="pp", bufs=2, space="PSUM"))

        # --- issue order on the single HWDGE stream (sync): first big
        # chunk pair immediately, then the small early-needed loads, then
        # the remaining interleaved big chunks.
        xts = []
        xhts = []

        def issue_pair(j):
            xt = xp.tile([P, MSE_CHUNKS[j][2]], f32, tag="xt")
            nc.sync.dma_start(xt[:], chunk(x_d, j))
            xts.append(xt)
            xht = xhp.tile([P, MSE_CHUNKS[j][2]], f32, tag="xht")
            nc.sync.dma_start(xht[:], chunk(xh_d, j))
            xhts.append(xht)

        N_EARLY = 2
        if "mse" in parts:
            for j in range(min(N_EARLY, NCH)):
                issue_pair(j)

        eye10 = st.tile([C, C], f32)
        nc.sync.dma_start(eye10[:], eye10_d[:])
        cenT = st.tile([P, C], f32)
        nc.sync.dma_start(cenT[:], ct_d[:])
        zf = st.tile([P, NT, ZF], f32)
        if parts & {"triplet", "outlier"}:
            nc.sync.dma_start(zf[:], zf_d[:])

        if "mse" in parts:
            for j in range(N_EARLY, NCH):
                issue_pair(j)

        # stats columns: [0:NCH] mse | [NCH:NCH+NT] tc |
        # [NCH+NT:NCH+2NT] outlier | [NCH+2NT] orth row-sums
        stats = st.tile([P, NSTAT], f32)
        nc.vector.memset(stats[:], 0.0)
        c_tc = NCH
        c_ol = NCH + NT
        c_or = NCH + 2 * NT

        # --- MSE: sum((x - x_hat)^2) for one chunk ---
        def mse_chunk(j):
            w = MSE_CHUNKS[j][2]
            df = dfp.tile([P, w], f32, tag="df")
            nc.vector.tensor_sub(df[:], xts[j][:], xhts[j][:])
            sq = sqp.tile([P, w], f32, tag="sq")
            nc.scalar.activation(
                sq[:], df[:], Act.Square, accum_out=stats[:, j : j + 1]
            )

        N_MSE_PRE = 2
        if "mse" in parts:
            for j in range(min(N_MSE_PRE, NCH)):
                mse_chunk(j)


        # --- orthogonality: gram = cenT.T @ cenT = cenN @ cenN.T ---
        if "orth" in parts:
            ps_g = pp.tile([C, C], f32)
            nc.tensor.matmul(ps_g[:], lhsT=cenT[:], rhs=cenT[:])
            gmi = st.tile([C, C], f32)
            nc.vector.tensor_sub(gmi[:], ps_g[:], eye10[:])
            gsc = st.tile([C, C], f32)
            nc.scalar.activation(
                gsc[:], gmi[:], Act.Square, accum_out=stats[0:C, c_or : c_or + 1]
            )

        # --- triplet-center loss terms ---
        if "triplet" in parts:
            for i in range(NT):
                z_nat = zf[:, i, 0:L]
                z_tr = zf[:, i, L : 2 * L]
                oh = zf[:, i, 3 * L : 3 * L + C]
                bm = zf[:, i, 3 * L + C : 3 * L + 2 * C]

                zscr = sp.tile([P, L], f32)
                zsq = sp.tile([P, 1], f32)
                nc.scalar.activation(
                    zscr[:], z_nat, Act.Square, accum_out=zsq[:]
                )
                zsq1 = sp.tile([P, 1], f32)
                nc.vector.tensor_scalar_add(zsq1[:], zsq[:], 1.0)

                # dot[b, c] = z . centers  (both operands pre-transposed)
                ps_dot = pp.tile([P, C], f32)
                nc.tensor.matmul(ps_dot[:], lhsT=z_tr, rhs=cenT[:])

                # d = sqrt(||z||^2 + 1 - 2 dot)  (centers are unit-norm)
                dd = sp.tile([P, C], f32)
                nc.scalar.activation(
                    dd[:], ps_dot[:], Act.Sqrt, scale=-2.0, bias=zsq1[:]
                )

                # pos = sum(d * onehot) = d[target];
                # negs = min over classes of (d - D_IN + bigmask)
                s1 = sp.tile([P, C], f32)
                pos = sp.tile([P, 1], f32)
                nc.vector.scalar_tensor_tensor(
                    out=s1[:],
                    in0=dd[:],
                    scalar=1.0,
                    in1=oh,
                    op0=Alu.mult,
                    op1=Alu.mult,
                    accum_out=pos[:],
                )
                s2 = sp.tile([P, C], f32)
                nc.vector.scalar_tensor_tensor(
                    out=s2[:],
                    in0=dd[:],
                    scalar=-D_IN,
                    in1=bm,
                    op0=Alu.add,
                    op1=Alu.add,
                )
                neg = sp.tile([P, 1], f32)
                nc.vector.tensor_reduce(
                    neg[:], s2[:], axis=mybir.AxisListType.X, op=Alu.min
                )
                v = sp.tile([P, 1], f32)
                nc.vector.tensor_sub(v[:], pos[:], neg[:])
                nc.scalar.activation(
                    stats[:, c_tc + i : c_tc + i + 1], v[:], Act.Relu
                )

        # --- outlier loss terms: relu(D_OUT - ||z_out||) ---
        if "outlier" in parts:
            for i in range(NT):
                zo_nat = zf[:, i, 2 * L : 3 * L]
                zos = sp.tile([P, L], f32)
                n2 = sp.tile([P, 1], f32)
                nc.scalar.activation(
                    zos[:], zo_nat, Act.Square, accum_out=n2[:]
                )
                nrm = sp.tile([P, 1], f32)
                nc.scalar.activation(nrm[:], n2[:], Act.Sqrt)
                nc.scalar.activation(
                    stats[:, c_ol + i : c_ol + i + 1],
                    nrm[:],
                    Act.Relu,
                    scale=-1.0,
                    bias=D_OUT,
                )


        if "mse" in parts:
            for j in range(N_MSE_PRE, NCH):
                mse_chunk(j)

        nc.sync.dma_start(out_d[:], stats[:])

    nc.compile()
    return nc


def _get_nc(parts=ALL_PARTS):
    key = ("nc", parts)
    if key not in _CACHE:
        _CACHE[key] = _build(parts)
    return _CACHE[key]


def _make_in_maps(inputs):
    x = np.ascontiguousarray(inputs["x"], dtype=np.float32)
    xh = np.ascontiguousarray(inputs["x_hat"], dtype=np.float32)
    zi = np.ascontiguousarray(inputs["z_in"], dtype=np.float32)
    zo = np.ascontiguousarray(inputs["z_out"], dtype=np.float32)
    tgt = np.asarray(inputs["target"]).astype(np.int64)
    cen = np.ascontiguousarray(inputs["center_arr"], dtype=np.float32)

    onehot = np.zeros((B, C), np.float32)
    onehot[np.arange(B), tgt] = 1.0
    bigmask = onehot * np.float32(BIG)

    norms = np.linalg.norm(cen, axis=1, keepdims=True).astype(np.float32)
    cen_n = (cen / norms).astype(np.float32)
    cen_t = np.ascontiguousarray(cen_n.T)

    in_maps = []
    for k in range(N_CORES):
        s = slice(k * BS, (k + 1) * BS)
        zi3 = zi[s].reshape(NT, P, L)
        zo3 = zo[s].reshape(NT, P, L)
        oh3 = onehot[s].reshape(NT, P, C)
        bm3 = bigmask[s].reshape(NT, P, C)
        zfused = np.concatenate(
            [
                zi3.transpose(1, 0, 2),  # z_nat [P, NT, L]
                zi3.transpose(2, 0, 1),  # z_tr  [L, NT, P]
                zo3.transpose(1, 0, 2),  # zo_nat [P, NT, L]
                oh3.transpose(1, 0, 2),  # onehot [P, NT, C]
                bm3.transpose(1, 0, 2),  # bigmask [P, NT, C]
            ],
            axis=-1,
        )
        in_maps.append(
            {
                "x": x[s],
                "x_hat": xh[s],
                "zfused": np.ascontiguousarray(zfused),
                "cen_t": cen_t,
            }
        )
    return in_maps


def _combine(results):
    outs = np.stack([np.asarray(r["out"], dtype=np.float64) for r in results])
    mse = outs[:, :, 0:NCH].sum() / (B * D)
    tcl = outs[:, :, NCH : NCH + NT].sum() / B
    ol = outs[:, :, NCH + NT : NCH + 2 * NT].sum() / B
    orth = np.sqrt(outs[0, :, NCH + 2 * NT].sum())
    return np.array(np.float32(mse + tcl + ol + orth))


def _run(inputs, trace=False, parts=ALL_PARTS):
    from concourse.bass_utils import run_bass_kernel_spmd

    nc = _get_nc(parts)
    in_maps = _make_in_maps(inputs)
    res = run_bass_kernel_spmd(nc, in_maps, core_ids=list(range(N_CORES)), trace=trace)
    return _combine(res.results), res.exec_time_ns


def kernel(**inputs):
    out, _ = _run(inputs, trace=False)
    return out


def run_traced(inputs):
    """For test.py: returns (output, hw exec_time_ns or None)."""
    return _run(inputs, trace=True)


# revision 17
# speedup vs baseline: 1.1204x; 1.1204x over previous
"""Self-contained Trainium2 Bass kernel for nn_CAELoss (loss_fn).

Contract: kernel(**inputs) takes the FULL unsharded inputs
(x [4096,3072], x_hat [4096,3072], target [4096] i32, z_in [4096,128],
z_out [4096,128], center_arr [10,128]) and returns the FULL output
(scalar f32 loss).

Strategy (data-parallel over batch, 8 NeuronCores):
  - each core gets 512 batch rows of x/x_hat/z_in/z_out (+ host-built
    one-hot masks of target), plus the replicated (host-normalized)
    centers.
  - on-device per core: partial sums of (x-x_hat)^2 [dominant, 12 MiB
    of DMA per core], triplet-center terms, outlier terms, and the
    orthogonality residual (gram - I) row sums.
  - device emits a [128, 17] tile of per-partition partial sums; host
    reduces the 8x128 partials to the scalar loss (replaces the
    all-reduce of scalar partial losses).
"""

import sys

import numpy as np

if "/opt/trn_rl_repo" not in sys.path:
    sys.path.insert(0, "/opt/trn_rl_repo")

B, D, C, L = 4096, 3072, 10, 128
N_CORES = 8
BS = B // N_CORES  # 512 batch rows per core
P = 128  # SBUF partitions
NT = BS // P  # 4 z-tiles of 128 rows per core
# MSE chunk layout: (row-tile, col offset, width); final row-tile tapers
# so the post-stream compute tail is short.
MSE_CHUNKS = [
    (0, 0, 1536), (0, 1536, 1536),
    (1, 0, 1536), (1, 1536, 1536),
    (2, 0, 1536), (2, 1536, 1536),
    (3, 0, 1536), (3, 1536, 768), (3, 2304, 768),
]
NCH = len(MSE_CHUNKS)
ZF = 3 * L + 2 * C  # fused z-row: z_nat | z_tr | zo_nat | oh | bm
NSTAT = NCH + NT + NT + 1  # stats columns: mse | tc | outlier | orth
D_IN = 0.1
D_OUT = 1.0
BIG = 1.0e9

ALL_PARTS = frozenset({"mse", "orth", "triplet", "outlier"})

_CACHE = {}


def _build(parts=ALL_PARTS):
    """Build + compile the single-core SPMD Bass program."""
    from contextlib import ExitStack

    import concourse.bacc as bacc
    import concourse.mybir as mybir
    import concourse.tile as tile

    f32 = mybir.dt.float32
    Alu = mybir.AluOpType
    Act = mybir.ActivationFunctionType

    nc = bacc.Bacc(
        "TRN2",
        target_bir_lowering=False,
        debug=False,
        enable_asserts=True,
        num_devices=N_CORES,
    )

    x_d = nc.dram_tensor("x", [BS, D], f32, kind="ExternalInput")
    xh_d = nc.dram_tensor("x_hat", [BS, D], f32, kind="ExternalInput")
    zf_d = nc.dram_tensor("zfused", [P, NT, ZF], f32, kind="ExternalInput")
    ct_d = nc.dram_tensor("cen_t", [L, C], f32, kind="ExternalInput")
    out_d = nc.dram_tensor("out", [P, NSTAT], f32, kind="ExternalOutput")

    eye10_d = nc.inline_tensor(np.eye(C, dtype=np.float32), "eye10")

    # chunk j -> (row-tile, col) slice of x/x_hat
    def chunk(td, j):
        r, c0, w = MSE_CHUNKS[j]
        return td[r * P : (r + 1) * P, c0 : c0 + w]

    with tile.TileContext(nc) as tc, ExitStack() as ctx:
        xp = ctx.enter_context(tc.tile_pool(name="xp", bufs=NCH))
        xhp = ctx.enter_context(tc.tile_pool(name="xhp", bufs=NCH))
        dfp = ctx.enter_context(tc.tile_pool(name="dfp", bufs=3))
        sqp = ctx.enter_context(tc.tile_pool(name="sqp", bufs=3))
        sp = ctx.enter_context(tc.tile_pool(name="sp", bufs=3))
        st = ctx.enter_context(tc.tile_pool(name="st", bufs=1))
        pp = ctx.enter_context(tc.tile_pool(name="pp", bufs=2, space="PSUM"))

        # --- issue order on the single HWDGE stream (sync): first big
        # chunk pair immediately, then the small early-needed loads, then
        # the remaining interleaved big chunks.
        xts = []
        xhts = []

        def issue_pair(j):
            xt = xp.tile([P, MSE_CHUNKS[j][2]], f32, tag="xt")
            nc.sync.dma_start(xt[:], chunk(x_d, j))
            xts.append(xt)
            xht = xhp.tile([P, MSE_CHUNKS[j][2]], f32, tag="xht")
            nc.sync.dma_start(xht[:], chunk(xh_d, j))
            xhts.append(xht)

        N_EARLY = 1
        if "mse" in parts:
            for j in range(min(N_EARLY, NCH)):
                issue_pair(j)

        eye10 = st.tile([C, C], f32)
        nc.sync.dma_start(eye10[:], eye10_d[:])
        cenT = st.tile([P, C], f32)
        nc.sync.dma_start(cenT[:], ct_d[:])
        zf = st.tile([P, NT, ZF], f32)
        if parts & {"triplet", "outlier"}:
            nc.sync.dma_start(zf[:], zf_d[:])

        if "mse" in parts:
            for j in range(N_EARLY, NCH):
                issue_pair(j)

        # stats columns: [0:NCH] mse | [NCH:NCH+NT] tc |
        # [NCH+NT:NCH+2NT] outlier | [NCH+2NT] orth row-sums
        stats = st.tile([P, NSTAT], f32)
        nc.vector.memset(stats[:], 0.0)
        c_tc = NCH
        c_ol = NCH + NT
        c_or = NCH + 2 * NT

        # --- MSE: sum((x - x_hat)^2) for one chunk ---
        def mse_chunk(j):
            w = MSE_CHUNKS[j][2]
            df = dfp.tile([P, w], f32, tag="df")
            nc.vector.tensor_sub(df[:], xts[j][:], xhts[j][:])
            sq = sqp.tile([P, w], f32, tag="sq")
            nc.scalar.activation(
                sq[:], df[:], Act.Square, accum_out=stats[:, j : j + 1]
            )

        N_MSE_PRE = 0
        if "mse" in parts:
            for j in range(min(N_MSE_PRE, NCH)):
                mse_chunk(j)


        # --- orthogonality: gram = cenT.T @ cenT = cenN @ cenN.T ---
        if "orth" in parts:
            ps_g = pp.tile([C, C], f32)
            nc.tensor.matmul(ps_g[:], lhsT=cenT[:], rhs=cenT[:])
            gmi = st.tile([C, C], f32)
            nc.vector.tensor_sub(gmi[:], ps_g[:], eye10[:])
            gsc = st.tile([C, C], f32)
            nc.scalar.activation(
                gsc[:], gmi[:], Act.Square, accum_out=stats[0:C, c_or : c_or + 1]
            )

        # --- triplet-center loss terms ---
        if "triplet" in parts:
            for i in range(NT):
                z_nat = zf[:, i, 0:L]
                z_tr = zf[:, i, L : 2 * L]
                oh = zf[:, i, 3 * L : 3 * L + C]
                bm = zf[:, i, 3 * L + C : 3 * L + 2 * C]

                zscr = sp.tile([P, L], f32)
                zsq = sp.tile([P, 1], f32)
                nc.scalar.activation(
                    zscr[:], z_nat, Act.Square, accum_out=zsq[:]
                )
                zsq1 = sp.tile([P, 1], f32)
                nc.vector.tensor_scalar_add(zsq1[:], zsq[:], 1.0)

                # dot[b, c] = z . centers  (both operands pre-transposed)
                ps_dot = pp.tile([P, C], f32)
                nc.tensor.matmul(ps_dot[:], lhsT=z_tr, rhs=cenT[:])

                # d = sqrt(||z||^2 + 1 - 2 dot)  (centers are unit-norm)
                dd = sp.tile([P, C], f32)
                nc.scalar.activation(
                    dd[:], ps_dot[:], Act.Sqrt, scale=-2.0, bias=zsq1[:]
                )

                # pos = sum(d * onehot) = d[target];
                # negs = min over classes of (d - D_IN + bigmask)
                s1 = sp.tile([P, C], f32)
                pos = sp.tile([P, 1], f32)
                nc.vector.scalar_tensor_tensor(
                    out=s1[:],
                    in0=dd[:],
                    scalar=1.0,
                    in1=oh,
                    op0=Alu.mult,
                    op1=Alu.mult,
                    accum_out=pos[:],
                )
                s2 = sp.tile([P, C], f32)
                nc.vector.scalar_tensor_tensor(
                    out=s2[:],
                    in0=dd[:],
                    scalar=-D_IN,
                    in1=bm,
                    op0=Alu.add,
                    op1=Alu.add,
                )
                neg = sp.tile([P, 1], f32)
                nc.vector.tensor_reduce(
                    neg[:], s2[:], axis=mybir.AxisListType.X, op=Alu.min
                )
                v = sp.tile([P, 1], f32)
                nc.vector.tensor_sub(v[:], pos[:], neg[:])
                nc.scalar.activation(
                    stats[:, c_tc + i : c_tc + i + 1], v[:], Act.Relu
                )

        # --- outlier loss terms: relu(D_OUT - ||z_out||) ---
        if "outlier" in parts:
            for i in range(NT):
                zo_nat = zf[:, i, 2 * L : 3 * L]
                zos = sp.tile([P, L], f32)
                n2 = sp.tile([P, 1], f32)
                nc.scalar.activation(
                    zos[:], zo_nat, Act.Square, accum_out=n2[:]
                )
                nrm = sp.tile([P, 1], f32)
                nc.scalar.activation(nrm[:], n2[:], Act.Sqrt)
                nc.scalar.activation(
                    stats[:, c_ol + i : c_ol + i + 1],
                    nrm[:],
                    Act.Relu,
                    scale=-1.0,
                    bias=D_OUT,
                )


        if "mse" in parts:
            for j in range(N_MSE_PRE, NCH):
                mse_chunk(j)

        nc.sync.dma_start(out_d[:], stats[:])

    nc.compile()
    return nc


def _get_nc(parts=ALL_PARTS):
    key = ("nc", parts)
    if key not in _CACHE:
        _CACHE[key] = _build(parts)
    return _CACHE[key]


def _make_in_maps(inputs):
    x = np.ascontiguousarray(inputs["x"], dtype=np.float32)
    xh = np.ascontiguousarray(inputs["x_hat"], dtype=np.float32)
    zi = np.ascontiguousarray(inputs["z_in"], dtype=np.float32)
    zo = np.ascontiguousarray(inputs["z_out"], dtype=np.float32)
    tgt = np.asarray(inputs["target"]).astype(np.int64)
    cen = np.ascontiguousarray(inputs["center_arr"], dtype=np.float32)

    onehot = np.zeros((B, C), np.float32)
    onehot[np.arange(B), tgt] = 1.0
    bigmask = onehot * np.float32(BIG)

    norms = np.linalg.norm(cen, axis=1, keepdims=True).astype(np.float32)
    cen_n = (cen / norms).astype(np.float32)
    cen_t = np.ascontiguousarray(cen_n.T)

    in_maps = []
    for k in range(N_CORES):
        s = slice(k * BS, (k + 1) * BS)
        zi3 = zi[s].reshape(NT, P, L)
        zo3 = zo[s].reshape(NT, P, L)
        oh3 = onehot[s].reshape(NT, P, C)
        bm3 = bigmask[s].reshape(NT, P, C)
        zfused = np.concatenate(
            [
                zi3.transpose(1, 0, 2),  # z_nat [P, NT, L]
                zi3.transpose(2, 0, 1),  # z_tr  [L, NT, P]
                zo3.transpose(1, 0, 2),  # zo_nat [P, NT, L]
                oh3.transpose(1, 0, 2),  # onehot [P, NT, C]
                bm3.transpose(1, 0, 2),  # bigmask [P, NT, C]
            ],
            axis=-1,
        )
        in_maps.append(
            {
                "x": x[s],
                "x_hat": xh[s],
                "zfused": np.ascontiguousarray(zfused),
                "cen_t": cen_t,
            }
        )
    return in_maps


def _combine(results):
    outs = np.stack([np.asarray(r["out"], dtype=np.float64) for r in results])
    mse = outs[:, :, 0:NCH].sum() / (B * D)
    tcl = outs[:, :, NCH : NCH + NT].sum() / B
    ol = outs[:, :, NCH + NT : NCH + 2 * NT].sum() / B
    orth = np.sqrt(outs[0, :, NCH + 2 * NT].sum())
    return np.array(np.float32(mse + tcl + ol + orth))


def _run(inputs, trace=False, parts=ALL_PARTS):
    from concourse.bass_utils import run_bass_kernel_spmd

    nc = _get_nc(parts)
    in_maps = _make_in_maps(inputs)
    res = run_bass_kernel_spmd(nc, in_maps, core_ids=list(range(N_CORES)), trace=trace)
    return _combine(res.results), res.exec_time_ns


def kernel(**inputs):
    out, _ = _run(inputs, trace=False)
    return out


def run_traced(inputs):
    """For test.py: returns (output, hw exec_time_ns or None)."""
    return _run(inputs, trace=True)
